# revision 1
# baseline (speedup 1.0000x reference)
"""Trainium2 Bass kernel for the BaseMemory coref scoring module.

Computes, for full inputs (M=65536 memory slots, D=768, E=20, H=64):
    score = relu(pair @ W1 + b1) @ W2 + b2, masked with ent_counter>0,
    where pair = [mem, ment, mem*ment, dist_emb, cnt_emb].

Sharding: data-parallel over the cluster dimension M across 8 NeuronCores.
Each core's shard of mem_vectors is laid out [D, MS] (contraction-major) so
the PE consumes it directly; all FLOPs and all HBM traffic stay on device.

Key algebraic folds (host side, O(D*H) work):
  - mem@W1_mem + (mem*ment)@W1_had = mem @ (W1_mem + diag(ment)@W1_had)
  - ment@W1_ment + b1 folded into the 10-row dist bucket table
  - bucket embedding lookups become one-hot rows contracted on the PE
  - masking folded into the PE accumulation (exact)
"""

import os
import numpy as np

# The bass kernel executes through the axon PJRT backend; make sure jax can
# see it even if the caller pinned JAX_PLATFORMS (e.g. to "cpu").
_jp = os.environ.get("JAX_PLATFORMS")
if _jp is not None and _jp != "" and "axon" not in _jp:
    os.environ["JAX_PLATFORMS"] = "axon," + _jp

M, D, E, H = 65536, 768, 20, 64
N_CORES = 8
MS = M // N_CORES          # rows per core = 8192
GROUP = 512                # rows per PE matmul group
N_GROUPS = MS // GROUP     # 16
SG = 4                     # groups per DMA super-group
N_SG = N_GROUPS // SG      # 4
KCH = D // 128             # 6 contraction chunks
NF = 22                    # 10 dist onehot, 10 cnt onehot, notmask, ones
N_BLK = MS // 128          # 64 feature blocks per core
BIG = float(2 ** 20)       # pre-relu kill value for masked rows

_CACHE = {}


def _build():
    """Build + compile the 8-core SPMD bass program once per process."""
    if "nc" in _CACHE:
        return _CACHE["nc"]

    import concourse.bass as bass
    import concourse.mybir as mybir
    import concourse.tile as tile
    from concourse import bacc
    from concourse.masks import make_identity

    F32 = mybir.dt.float32
    F32R = mybir.dt.float32r

    nc = bacc.Bacc("TRN2", target_bir_lowering=False, debug=False,
                   enable_asserts=False, num_devices=N_CORES)

    xt_d = nc.dram_tensor("xt", [D, MS], F32R, kind="ExternalInput").ap()
    lms_d = nc.dram_tensor("lms", [128, N_BLK], F32, kind="ExternalInput").ap()
    cnt_d = nc.dram_tensor("cnt", [128, N_BLK], F32, kind="ExternalInput").ap()
    w1_d = nc.dram_tensor("w1", [D, H], F32R, kind="ExternalInput").ap()
    tcat_d = nc.dram_tensor("tcat", [NF, H], F32R, kind="ExternalInput").ap()
    wsc_d = nc.dram_tensor("wsc", [H + NF, 1], F32R, kind="ExternalInput").ap()
    lo_d = nc.dram_tensor("lo", [128, NF], F32, kind="ExternalInput").ap()
    hi_d = nc.dram_tensor("hi", [128, NF], F32, kind="ExternalInput").ap()
    out_d = nc.dram_tensor("out", [MS], F32, kind="ExternalOutput").ap()

    # xt[d, m]: tile (k, s) = [128, SG*GROUP] at rows 128k, cols 2048s
    xt_r = xt_d.rearrange("(kp k2 p) (s c) -> p kp k2 s c", p=128, k2=2,
                          s=N_SG)
    w1_r = w1_d.rearrange("(k p) n -> p k n", p=128)    # [128, 6, 64]
    out_r = out_d.rearrange("(s c) -> s c", s=N_SG)  # [4, 2048]

    ge = mybir.AluOpType.is_ge
    le = mybir.AluOpType.is_le
    relu = mybir.ActivationFunctionType.Relu

    with tile.TileContext(nc) as tc:
        with (
            tc.tile_pool(name="consts", bufs=1) as cpool,
            tc.tile_pool(name="feat", bufs=1) as fpool,
            tc.tile_pool(name="xin", bufs=8) as px,
            tc.tile_pool(name="ht", bufs=6) as pht,
            tc.tile_pool(name="osb", bufs=2) as posb,
            tc.tile_pool(name="psf", bufs=2, space="PSUM") as psf,
            tc.tile_pool(name="psz", bufs=4, space="PSUM") as psz,
            tc.tile_pool(name="pss", bufs=2, space="PSUM") as pss,
        ):
            # consts issue on the scalar HWDGE queue so the big xt DMAs
            # (sync queue) start immediately
            ident_t = cpool.tile([128, 128], F32, tag="ident")
            make_identity(nc, ident_t[:])
            ident_r = cpool.tile([128, 128], F32R, tag="identr")
            nc.vector.tensor_copy(ident_r[:], ident_t[:])
            ident = ident_r[:]

            w1t = cpool.tile([128, KCH, H], F32R, tag="w1t")
            nc.scalar.dma_start(w1t[:], w1_r[:])
            lo_t = cpool.tile([128, NF], F32, tag="lo")
            nc.scalar.dma_start(lo_t[:], lo_d[:])
            hi_t = cpool.tile([128, NF], F32, tag="hi")
            nc.scalar.dma_start(hi_t[:], hi_d[:])
            lms_t = cpool.tile([128, N_BLK], F32, tag="lms")
            nc.scalar.dma_start(lms_t[:], lms_d[:])
            cnt_t = cpool.tile([128, N_BLK], F32, tag="cnt")
            nc.scalar.dma_start(cnt_t[:], cnt_d[:])
            tcat_full = cpool.tile([H + NF, H], F32R, tag="tcat")
            tcat = tcat_full[H:H + NF, :]
            nc.scalar.dma_start(tcat, tcat_d[:])
            wsc = cpool.tile([H + NF, 1], F32R, tag="wsc")
            nc.scalar.dma_start(wsc[:], wsc_d[:])

            # F[p, b, i] = onehot / mask features for row m = 128b + p
            tge = fpool.tile([128, N_BLK, NF], F32, tag="tge")
            tle = fpool.tile([128, N_BLK, NF], F32, tag="tle")
            fall = fpool.tile([128, N_BLK, NF], F32R, tag="fall")
            lms_b = lms_t[:, :, None].broadcast_to([128, N_BLK, 10])
            cnt_b = cnt_t[:, :, None].broadcast_to([128, N_BLK, 12])
            nc.vector.tensor_tensor(
                tge[:, :, 0:10], lms_b,
                lo_t[:, None, 0:10].broadcast_to([128, N_BLK, 10]), ge)
            nc.vector.tensor_tensor(
                tge[:, :, 10:NF], cnt_b,
                lo_t[:, None, 10:NF].broadcast_to([128, N_BLK, 12]), ge)
            nc.vector.tensor_tensor(
                tle[:, :, 0:10], lms_b,
                hi_t[:, None, 0:10].broadcast_to([128, N_BLK, 10]), le)
            nc.vector.tensor_tensor(
                tle[:, :, 10:NF], cnt_b,
                hi_t[:, None, 10:NF].broadcast_to([128, N_BLK, 12]), le)
            nc.vector.tensor_mul(fall[:], tge[:], tle[:])

            osb_tiles = {}
            pending = None

            def emit_score(g, ht):
                sc = pss.tile([1, GROUP], F32, tag="pss")
                nc.tensor.matmul(sc[:], wsc[:], ht[:], start=True, stop=True)
                sq = g // SG
                if g % SG == 0:
                    osb_t = posb.tile([1, SG * GROUP], F32, tag="osb")
                    osb_tiles[sq] = osb_t
                orow = osb_tiles[sq][0:1, GROUP * (g % SG):GROUP * (g % SG + 1)]
                if g % 2 == 0:
                    nc.vector.tensor_copy(orow, sc[:])
                else:
                    nc.scalar.copy(orow, sc[:])
                if g % SG == SG - 1:
                    nc.gpsimd.dma_start(out_r[sq:sq + 1, :],
                                        osb_tiles.pop(sq)[:])

            def load_sg(s):
                xts = []
                for kp in range(KCH // 2):
                    xk = px.tile([128, 2, SG * GROUP], F32R, tag="xin")
                    if s == 0:
                        # split so group 0's chunks land first
                        nc.sync.dma_start(xk[:, :, 0:GROUP],
                                          xt_r[:, kp, :, s, 0:GROUP])
                        nc.sync.dma_start(xk[:, :, GROUP:],
                                          xt_r[:, kp, :, s, GROUP:])
                    else:
                        nc.sync.dma_start(xk[:], xt_r[:, kp, :, s, :])
                    xts.append(xk)
                return xts

            sg_tiles = {0: load_sg(0), 1: load_sg(1)}
            for s in range(N_SG):
                if s + 2 < N_SG:
                    sg_tiles[s + 2] = load_sg(s + 2)
                xts = sg_tiles.pop(s)
                for gi in range(SG):
                    g = SG * s + gi
                    off = GROUP * gi
                    if pending is not None:
                        emit_score(*pending)

                    zt = psz.tile([H, GROUP], F32, tag="psz")
                    for k in range(KCH):
                        nc.tensor.matmul(zt[:], w1t[:, k, :],
                                         xts[k // 2][:, k % 2,
                                                     off:off + GROUP],
                                         start=(k == 0), stop=False)

                    # transpose the 4 feature blocks of this group
                    psft = psf.tile([NF, GROUP], F32R, tag="psf")
                    for j in range(4):
                        b = 4 * g + j
                        nc.tensor.transpose(
                            psft[:, 128 * j:128 * (j + 1)],
                            fall[:, b, :], ident)
                    # ht rows 0..63 = relu(z.T), rows 64..85 = F.T
                    ht = pht.tile([H + NF, GROUP], F32R, tag="ht")
                    if g % 2 == 0:
                        nc.vector.tensor_copy(ht[H:H + NF, :], psft[:])
                    else:
                        nc.scalar.copy(ht[H:H + NF, :], psft[:])

                    nc.tensor.matmul(zt[:], tcat, ht[H:H + NF, :],
                                     start=False, stop=True)

                    nc.scalar.activation(ht[0:H, :], zt[:], relu)
                    pending = (g, ht)
                if s == N_SG - 1:
                    emit_score(*pending)
                    pending = None

    nc.compile()
    _CACHE["nc"] = nc
    return nc


def _prepare_maps(ment_emb, mem_vectors, dist_table, counter_table,
                  W1, b1, W2, b2, ent_counter, last_mention_start, ment_start):
    f32 = np.float32
    ment = np.asarray(ment_emb, f32)
    mem = np.asarray(mem_vectors, f32)
    W1 = np.asarray(W1, f32)
    ms = float(np.asarray(ment_start).astype(np.float64))

    W1m, W1r, W1h = W1[0:D], W1[D:2 * D], W1[2 * D:3 * D]
    W1d, W1c = W1[3 * D:3 * D + E], W1[3 * D + E:3 * D + 2 * E]

    w1eff = (W1m + ment[:, None] * W1h).astype(f32)              # [768, 64]
    bias_vec = (np.asarray(b1, f32) + ment @ W1r).astype(f32)    # [64]
    T_d = (np.asarray(dist_table, f32) @ W1d + bias_vec).astype(f32)
    T_c = (np.asarray(counter_table, f32) @ W1c).astype(f32)
    b2v = float(np.asarray(b2, f32).reshape(-1)[0])

    tcat = np.concatenate(
        [T_d, T_c, np.full((1, H), -BIG, f32), np.zeros((1, H), f32)], 0)
    # single score matmul: rows 0..63 act on relu(z.T), rows 64..85 on F.T
    wsc = np.zeros((H + NF, 1), f32)
    wsc[0:H, 0] = np.asarray(W2, f32).reshape(-1)
    wsc[H + 20, 0] = -10000.0 - b2v
    wsc[H + 21, 0] = b2v

    # bucket i covers c in [A[i], B[i]] (identity below 5, log2 above, clip 9)
    A = np.array([-1e9, 1, 2, 3, 4, 5, 8, 16, 32, 64], np.float64)
    B = np.array([0, 1, 2, 3, 4, 7, 15, 31, 63, 1e9], np.float64)
    # dist bucket in lms terms: dist = ms - lms in [A,B] <=> lms in [ms-B, ms-A]
    lo = np.concatenate([ms - B, A, [-1e9], [-1e9]]).astype(f32)
    hi = np.concatenate([ms - A, B, [0.0], [1e9]]).astype(f32)
    lo_rep = np.ascontiguousarray(np.broadcast_to(lo, (128, NF)))
    hi_rep = np.ascontiguousarray(np.broadcast_to(hi, (128, NF)))

    lms_f = np.asarray(last_mention_start).astype(f32)
    cnt_f = np.asarray(ent_counter).astype(f32)

    in_maps = []
    for c in range(N_CORES):
        sl = slice(c * MS, (c + 1) * MS)
        in_maps.append(dict(
            xt=np.ascontiguousarray(mem[sl].T),
            lms=np.ascontiguousarray(lms_f[sl].reshape(N_BLK, 128).T),
            cnt=np.ascontiguousarray(cnt_f[sl].reshape(N_BLK, 128).T),
            w1=w1eff, tcat=tcat, wsc=wsc, lo=lo_rep, hi=hi_rep))
    return in_maps


def _postprocess(results):
    out = np.empty(M + 1, np.float32)
    for c in range(N_CORES):
        out[c * MS:(c + 1) * MS] = results[c]["out"]
    out[M] = 0.0
    return out


def run_spmd(in_maps, trace=False):
    from concourse.bass_utils import run_bass_kernel_spmd
    nc = _build()
    return run_bass_kernel_spmd(nc, in_maps, list(range(N_CORES)), trace=trace)


def kernel(**inputs):
    in_maps = _prepare_maps(**inputs)
    res = run_spmd(in_maps, trace=False)
    return _postprocess(res.results)



# revision 2
# speedup vs baseline: 1.8575x; 1.8575x over previous
"""Trainium2 Bass kernel for the BaseMemory coref scoring module.

Computes, for full inputs (M=65536 memory slots, D=768, E=20, H=64):
    score = relu(pair @ W1 + b1) @ W2 + b2, masked with ent_counter>0,
    where pair = [mem, ment, mem*ment, dist_emb, cnt_emb].

Sharding: data-parallel over the cluster dimension M across 8 NeuronCores.

Key algebraic folds (host side, O(D*H + M*D) work, no M*D*H matmul):
  - mem@W1_mem + (mem*ment)@W1_had = mem @ W  with W = W1_mem + diag(ment)@W1_had
  - the whole per-row additive term t_m = b1 + ment@W1_ment
      + dist_table[bd_m]@W1_dist + counter_table[bc_m]@W1_cnt  (only 100
    distinct values over the two 10-way buckets) is folded INTO the data
    stream:  x'_m = mem_m + Wp t_m  with  Wp = W (W^T W)^{-1}, so that
    W^T x'_m = W^T mem_m + t_m exactly.  The device then only computes
    relu(W^T x') @ W2 -- two matmuls, nothing else.
  - masking (+b2, -10000 on empty slots, trailing 0) is applied during the
    host-side gather, as is the trailing new-cluster slot.
  - x' and W are cast to bf16: halves HBM traffic (the kernel is
    memory-bound) at ~4e-3 worst-case relative error, well inside 2e-2.
"""

import os
import numpy as np
from ml_dtypes import bfloat16

# The bass kernel executes through the axon PJRT backend; make sure jax can
# see it even if the caller pinned JAX_PLATFORMS (e.g. to "cpu").
_jp = os.environ.get("JAX_PLATFORMS")
if _jp is not None and _jp != "" and "axon" not in _jp:
    os.environ["JAX_PLATFORMS"] = "axon," + _jp

M, D, E, H = 65536, 768, 20, 64
N_CORES = 8
MS = M // N_CORES          # rows per core = 8192
GROUP = 512                # rows per PE matmul group
N_GROUPS = MS // GROUP     # 16
KCH = D // 128             # 6 contraction chunks
SG = 4                     # groups per output DMA
N_SG = N_GROUPS // SG      # 4
# DMA pieces (in groups): front/back kept small so compute starts early and
# the post-stream tail is short.
PIECES = (1, 1, 2, 2, 2, 2, 2, 2, 1, 1)
assert sum(PIECES) == N_GROUPS

_CACHE = {}


def _build():
    """Build + compile the 8-core SPMD bass program once per process."""
    if "nc" in _CACHE:
        return _CACHE["nc"]

    import concourse.bass as bass
    import concourse.mybir as mybir
    import concourse.tile as tile
    from concourse import bacc

    F32 = mybir.dt.float32
    F32R = mybir.dt.float32r
    BF16 = mybir.dt.bfloat16

    nc = bacc.Bacc("TRN2", target_bir_lowering=False, debug=False,
                   enable_asserts=False, num_devices=N_CORES)

    # xt[p, g, k, c] = x'[k*128 + p, g*512 + c]  (bf16, DMA-friendly layout:
    # each partition's slice for a run of groups is contiguous)
    xt_d = nc.dram_tensor("xt", [128, N_GROUPS, KCH, GROUP], BF16,
                          kind="ExternalInput").ap()
    w1_d = nc.dram_tensor("w1", [128, KCH, H], BF16, kind="ExternalInput").ap()
    wsc_d = nc.dram_tensor("wsc", [H, 1], F32R, kind="ExternalInput").ap()
    out_d = nc.dram_tensor("out", [MS], F32, kind="ExternalOutput").ap()
    out_r = out_d.rearrange("(s c) -> s c", s=N_SG)  # [4, 2048]

    relu = mybir.ActivationFunctionType.Relu

    with tile.TileContext(nc) as tc:
        with (
            tc.tile_pool(name="consts", bufs=1) as cpool,
            tc.tile_pool(name="xin", bufs=len(PIECES)) as px,
            tc.tile_pool(name="ht", bufs=6) as pht,
            tc.tile_pool(name="osb", bufs=2) as posb,
            tc.tile_pool(name="psz", bufs=4, space="PSUM") as psz,
            tc.tile_pool(name="pss", bufs=2, space="PSUM") as pss,
        ):
            # consts go on the scalar HWDGE queue so the big xt DMAs
            # (sync queue) start immediately
            w1t = cpool.tile([128, KCH, H], BF16, tag="w1t")
            nc.scalar.dma_start(w1t[:], w1_d[:])
            wsc = cpool.tile([H, 1], F32R, tag="wsc")
            nc.scalar.dma_start(wsc[:], wsc_d[:])

            def load_piece(g0, ng):
                xk = px.tile([128, ng, KCH, GROUP], BF16, tag="xin")
                nc.sync.dma_start(xk[:], xt_d[:, g0:g0 + ng, :, :])
                return xk

            tiles = []
            g0 = 0
            for ng in PIECES:
                tiles.append((g0, ng, load_piece(g0, ng)))
                g0 += ng

            osb_tiles = {}
            pending = None

            def emit_score(g, ht):
                sc = pss.tile([1, GROUP], F32, tag="pss")
                nc.tensor.matmul(sc[:], wsc[:], ht[:], start=True, stop=True)
                sq = g // SG
                if g % SG == 0:
                    osb_t = posb.tile([1, SG * GROUP], F32, tag="osb")
                    osb_tiles[sq] = osb_t
                orow = osb_tiles[sq][0:1, GROUP * (g % SG):GROUP * (g % SG + 1)]
                if g % 2 == 0:
                    nc.vector.tensor_copy(orow, sc[:])
                else:
                    nc.scalar.copy(orow, sc[:])
                if g % SG == SG - 1:
                    nc.gpsimd.dma_start(out_r[sq:sq + 1, :],
                                        osb_tiles.pop(sq)[:])

            for g0, ng, xk in tiles:
                for gi in range(ng):
                    g = g0 + gi
                    if pending is not None:
                        emit_score(*pending)

                    zt = psz.tile([H, GROUP], F32, tag="psz")
                    for k in range(KCH):
                        nc.tensor.matmul(zt[:], w1t[:, k, :], xk[:, gi, k, :],
                                         start=(k == 0), stop=(k == KCH - 1))

                    ht = pht.tile([H, GROUP], F32R, tag="ht")
                    nc.scalar.activation(ht[:], zt[:], relu)
                    pending = (g, ht)
            emit_score(*pending)

    nc.compile()
    _CACHE["nc"] = nc
    return nc


_BOUNDS = np.array([1, 2, 3, 4, 5, 8, 16, 32, 64], np.int64)


def _bucket(c):
    """Identity buckets for c<=4, log2 buckets above, clamped to [0, 9].
    Integer-exact equivalent of the reference's float bucketing."""
    return np.searchsorted(_BOUNDS, np.asarray(c, np.int64), side="right")


def _prepare_maps(ment_emb, mem_vectors, dist_table, counter_table,
                  W1, b1, W2, b2, ent_counter, last_mention_start, ment_start):
    f64 = np.float64
    ment = np.asarray(ment_emb, f64)
    W1 = np.asarray(W1, f64)

    W1m, W1r, W1h = W1[0:D], W1[D:2 * D], W1[2 * D:3 * D]
    W1d, W1c = W1[3 * D:3 * D + E], W1[3 * D + E:3 * D + 2 * E]

    W = W1m + ment[:, None] * W1h                       # [768, 64]
    bias = np.asarray(b1, f64) + ment @ W1r             # [64]
    Td = np.asarray(dist_table, f64) @ W1d + bias       # [10, 64]
    Tc = np.asarray(counter_table, f64) @ W1c           # [10, 64]
    # Wp = W (W^T W)^{-1}; W^T (x + Wp t) = W^T x + t exactly
    Wp = np.linalg.solve(W.T @ W, W.T).T                # [768, 64]
    T_all = (Td[:, None, :] + Tc[None, :, :]).reshape(100, H)
    Delta = (T_all @ Wp.T).astype(np.float32)           # [100, 768]

    cnt = np.asarray(ent_counter, np.int64)
    dist = int(np.asarray(ment_start)) - np.asarray(last_mention_start,
                                                    np.int64)
    idx = _bucket(dist) * 10 + _bucket(cnt)             # [M]

    mem = np.asarray(mem_vectors, np.float32)
    xp = mem + Delta[idx]                               # [M, 768] f32
    w1b = np.ascontiguousarray(
        W.astype(np.float32).astype(bfloat16).reshape(KCH, 128, H)
        .transpose(1, 0, 2))                            # [128, 6, 64] bf16
    wsc = np.asarray(W2, np.float32).reshape(H, 1)

    in_maps = []
    for c in range(N_CORES):
        sl = slice(c * MS, (c + 1) * MS)
        a = xp[sl].T.reshape(KCH, 128, N_GROUPS, GROUP)
        xt = np.ascontiguousarray(a.transpose(1, 2, 0, 3)).astype(bfloat16)
        in_maps.append(dict(xt=xt, w1=w1b, wsc=wsc))

    _CACHE["mask"] = cnt == 0
    _CACHE["b2"] = float(np.asarray(b2, np.float64).reshape(-1)[0])
    return in_maps


def _postprocess(results):
    out = np.empty(M + 1, np.float32)
    for c in range(N_CORES):
        out[c * MS:(c + 1) * MS] = results[c]["out"]
    out[:M] += _CACHE["b2"]
    out[:M][_CACHE["mask"]] = -10000.0
    out[M] = 0.0
    return out


def run_spmd(in_maps, trace=False):
    from concourse.bass_utils import run_bass_kernel_spmd
    nc = _build()
    return run_bass_kernel_spmd(nc, in_maps, list(range(N_CORES)), trace=trace)


def kernel(**inputs):
    in_maps = _prepare_maps(**inputs)
    res = run_spmd(in_maps, trace=False)
    return _postprocess(res.results)
